# revision 4
# baseline (speedup 1.0000x reference)
"""Trainium2 Bass kernel for AttentionWeightNet (MLP -> attention logits -> top-64 -> softmax).

Strategy (8 NeuronCores, data-parallel over batch):
  - Each core handles 512 of the 4096 batch rows. MLP params + 16MB key bank replicated.
  - Everything feature-major on device: activations stored [features, batch] so matmul
    contraction (K) sits on partitions and weights are naturally [K, M] (= lhsT).
  - Logits tiles [128 rows x 512 keys] are produced by PE into PSUM; the DVE max8
    instruction extracts each tile's top-8 values and max_index their within-tile
    positions (top-64 of a row provably lives in the per-512-chunk top-8 unless a
    single 512-chunk holds >=9 of the row's top-64 -- probability ~2e-3 per full
    problem instance, verified false on the reference input distribution).
  - Stage 2: per row-block, 8 rounds of (max8 + match_replace) over the 1024
    candidates give the sorted top-64 values; max_index recovers candidate slots;
    softmax on ACT+DVE. Host maps (slot -> tile, within-tile idx) to global indices
    (pure layout gather, no arithmetic).
"""

import math
import numpy as np

import concourse.bass as bass
import concourse.mybir as mybir
import concourse.tile as tile
from concourse import bacc
from concourse.bass_utils import run_bass_kernel_spmd

NCORES = 8
B, XD, TD, HID, KD, N, TOPK = 4096, 128, 32, 512, 64, 65536, 64
BC = B // NCORES          # rows per core
NBLK = BC // 128          # row blocks of 128 per core
TS = 512                  # keys per logits tile
NT = N // TS              # logits tiles per row block
KCH = 4096                # keys per DMA chunk
NKC = N // KCH            # key chunks
CAND = NT * 8             # stage-2 candidates per row
SELU_ALPHA = 1.6732632423543772
SELU_SCALE = 1.0507009873554805

F32 = mybir.dt.float32
U16 = mybir.dt.uint16
AF = mybir.ActivationFunctionType
ALU = mybir.AluOpType
NEG_BIG = -1.0e30

_CACHE = {}


def _build_program():
    nc = bacc.Bacc(
        "TRN2",
        target_bir_lowering=False,
        debug=False,
        enable_asserts=False,
        num_devices=NCORES,
    )

    def din(name, shape, dt=F32):
        return nc.dram_tensor(name, shape, dt, kind="ExternalInput").ap()

    def dout(name, shape, dt=F32):
        return nc.dram_tensor(name, shape, dt, kind="ExternalOutput").ap()

    ins = {
        "x_fm": din("x_fm", [XD, BC]),
        "temb_fm": din("temb_fm", [TD, BC]),
        "w0x": din("w0x", [XD, HID]),
        "w0t": din("w0t", [TD, HID]),
        "w1": din("w1", [128, 4 * HID]),
        "w2": din("w2", [128, 4 * HID]),
        "wq": din("wq", [128, 4 * KD]),
        "b0": din("b0", [128, 4]),
        "b1": din("b1", [128, 4]),
        "b2": din("b2", [128, 4]),
        "bq": din("bq", [KD, 1]),
        "keys_fm": din("keys_fm", [KD, N]),
    }
    outs = {
        "weights_o": dout("weights_o", [BC, TOPK]),
        "pos_o": dout("pos_o", [BC, TOPK], U16),
        "iw_o": dout("iw_o", [BC, CAND], U16),
    }

    with tile.TileContext(nc) as tc:
        _emit(tc, ins, outs)
    nc.compile()
    return nc


def _emit(tc, ins, outs):
    nc = tc.nc

    cp = tc.alloc_tile_pool(name="const", bufs=1)
    wp = tc.alloc_tile_pool(name="work", bufs=2)
    sp = tc.alloc_tile_pool(name="stage2", bufs=2)
    keys_pool = tc.alloc_tile_pool(name="keys", bufs=2)

    def load_const(name, shape, dt=F32):
        t = cp.tile(shape, dt, tag=name)
        nc.sync.dma_start(t[:], ins[name])
        return t

    x_fm = load_const("x_fm", [XD, BC])
    temb = load_const("temb_fm", [TD, BC])
    w0x = load_const("w0x", [XD, HID])
    w0t = load_const("w0t", [TD, HID])
    w1 = load_const("w1", [128, 4 * HID])
    w2 = load_const("w2", [128, 4 * HID])
    wq = load_const("wq", [128, 4 * KD])
    b0 = load_const("b0", [128, 4])
    b1 = load_const("b1", [128, 4])
    b2 = load_const("b2", [128, 4])
    bq = load_const("bq", [KD, 1])

    h1 = cp.tile([128, 4 * BC], F32, tag="h1")
    h2 = cp.tile([128, 4 * BC], F32, tag="h2")
    h3 = cp.tile([128, 4 * BC], F32, tag="h3")
    q = cp.tile([KD, BC], F32, tag="q")

    # ---- MLP (feature-major) ----
    with tc.tile_pool(name="psum_mlp", bufs=2, space="PSUM") as pmlp:

        def selu_from_psum(ps, out_ap, b_col):
            # selu(x+b) with x in PSUM; s*(x if x>0 else alpha*(exp(x)-1))
            e = wp.tile([128, BC], F32, tag="selu_e")
            xb = wp.tile([128, BC], F32, tag="selu_x")
            nc.scalar.activation(e[:], ps, AF.Exp, bias=b_col, scale=1.0)
            nc.scalar.activation(xb[:], ps, AF.Identity, bias=b_col, scale=1.0)
            # e <- s*alpha*(min(e,1) - 1)
            nc.vector.tensor_scalar(
                e[:], e[:], 1.0, None, op0=ALU.min
            )
            nc.vector.tensor_scalar(
                e[:], e[:], SELU_SCALE * SELU_ALPHA, -SELU_SCALE * SELU_ALPHA,
                op0=ALU.mult, op1=ALU.add,
            )
            # xb <- s*max(xb, 0)
            nc.vector.tensor_scalar(
                xb[:], xb[:], 0.0, SELU_SCALE, op0=ALU.max, op1=ALU.mult
            )
            nc.vector.tensor_add(out_ap, xb[:], e[:])

        # layer 1: in = [x (K=128); temb (K=32)]
        for m in range(4):
            ps = pmlp.tile([128, BC], F32, tag="mlp")
            nc.tensor.matmul(
                ps[:], w0x[:, m * 128:(m + 1) * 128], x_fm[:], start=True, stop=False
            )
            nc.tensor.matmul(
                ps[:], w0t[:, m * 128:(m + 1) * 128], temb[:], start=False, stop=True
            )
            selu_from_psum(ps[:], h1[:, m * BC:(m + 1) * BC], b0[:, m:m + 1])

        # layers 2 and 3: K = 512 in 4 chunks
        for (win, bin_, hin, hout) in ((w1, b1, h1, h2), (w2, b2, h2, h3)):
            for m in range(4):
                ps = pmlp.tile([128, BC], F32, tag="mlp")
                for k in range(4):
                    nc.tensor.matmul(
                        ps[:],
                        win[:, k * HID + m * 128: k * HID + (m + 1) * 128],
                        hin[:, k * BC:(k + 1) * BC],
                        start=(k == 0),
                        stop=(k == 3),
                    )
                selu_from_psum(ps[:], hout[:, m * BC:(m + 1) * BC], bin_[:, m:m + 1])

        # q = (h3 @ Wq + bq) / 8   (logit scale folded in; /8 is exact)
        psq = pmlp.tile([KD, BC], F32, tag="qp")
        for k in range(4):
            nc.tensor.matmul(
                psq[:],
                wq[:, k * KD:(k + 1) * KD],
                h3[:, k * BC:(k + 1) * BC],
                start=(k == 0),
                stop=(k == 3),
            )
        nc.scalar.activation(q[:], psq[:], AF.Identity, bias=bq[:, 0:1], scale=0.125)

    # ---- logits + per-tile top-8 screening ----
    W = [cp.tile([128, CAND], F32, name=f"W{b}", tag=f"W{b}") for b in range(NBLK)]
    Iw = [cp.tile([128, CAND], U16, name=f"I{b}", tag=f"I{b}") for b in range(NBLK)]

    with tc.tile_pool(name="psum_lg", bufs=8, space="PSUM") as plg:
        for kc in range(NKC):
            kt = keys_pool.tile([KD, KCH], F32, tag="keys")
            nc.sync.dma_start(kt[:], ins["keys_fm"][:, kc * KCH:(kc + 1) * KCH])
            for blk in range(NBLK):
                lhsT = q[:, blk * 128:(blk + 1) * 128]
                for t in range(KCH // TS):
                    ti = kc * (KCH // TS) + t
                    ps = plg.tile([128, TS], F32, tag="lg")
                    nc.tensor.matmul(
                        ps[:], lhsT, kt[:, t * TS:(t + 1) * TS], start=True, stop=True
                    )
                    wsl = W[blk][:, ti * 8:(ti + 1) * 8]
                    nc.vector.max(wsl, ps[:])
                    nc.vector.max_index(Iw[blk][:, ti * 8:(ti + 1) * 8], wsl, ps[:])

    # ---- stage 2: top-64 of the 1024 candidates, then softmax ----
    for blk in range(NBLK):
        Wo = W[blk]
        Wk = sp.tile([128, CAND], F32, tag="wwork")
        R = sp.tile([128, TOPK], F32, tag="rvals")
        for r in range(8):
            src = Wo[:] if r == 0 else Wk[:]
            rsl = R[:, r * 8:(r + 1) * 8]
            nc.vector.max(rsl, src)
            nc.vector.match_replace(Wk[:], rsl, src, NEG_BIG)
        pos = sp.tile([128, TOPK], U16, tag="pos")
        for r in range(8):
            nc.vector.max_index(pos[:, r * 8:(r + 1) * 8], R[:, r * 8:(r + 1) * 8], Wo[:])

        negm = sp.tile([128, 1], F32, tag="negm")
        nc.vector.tensor_scalar(negm[:], R[:, 0:1], -1.0, None, op0=ALU.mult)
        e64 = sp.tile([128, TOPK], F32, tag="e64")
        nc.scalar.activation(e64[:], R[:], AF.Exp, bias=negm[:, 0:1], scale=1.0)
        ssum = sp.tile([128, 1], F32, tag="ssum")
        nc.vector.reduce_sum(ssum[:], e64[:], axis=mybir.AxisListType.X)
        rec = sp.tile([128, 1], F32, tag="rec")
        nc.vector.reciprocal(rec[:], ssum[:])
        wts = sp.tile([128, TOPK], F32, tag="wts")
        nc.vector.tensor_mul(wts[:], e64[:], rec[:, 0:1].to_broadcast([128, TOPK]))

        rows = slice(blk * 128, (blk + 1) * 128)
        nc.sync.dma_start(outs["weights_o"][rows, :], wts[:])
        nc.sync.dma_start(outs["pos_o"][rows, :], pos[:])
        nc.sync.dma_start(outs["iw_o"][rows, :], Iw[blk][:])

    for pool in (keys_pool, sp, wp, cp):
        pool.release()


def _prep_inputs(x, t, W0, b0, W1, b1, W2, b2, Wq, bq, keys):
    """Host-side layout marshaling (transposes/reshapes only; temb is the one
    tiny precompute: 4096x16 sin/cos)."""
    f = np.float32
    half = TD // 2
    freqs = np.exp(-math.log(10000.0) * np.arange(half, dtype=np.float64) / half)
    args = t.astype(np.float64)[:, None] * freqs[None, :]
    temb = np.concatenate([np.sin(args), np.cos(args)], axis=-1).astype(f)  # [B, 32]

    com = {
        "w0x": np.ascontiguousarray(W0[:XD, :], dtype=f),
        "w0t": np.ascontiguousarray(W0[XD:, :], dtype=f),
        "w1": np.ascontiguousarray(
            W1.reshape(4, 128, HID).transpose(1, 0, 2).reshape(128, 4 * HID), dtype=f
        ),
        "w2": np.ascontiguousarray(
            W2.reshape(4, 128, HID).transpose(1, 0, 2).reshape(128, 4 * HID), dtype=f
        ),
        "wq": np.ascontiguousarray(
            Wq.reshape(4, 128, KD).transpose(1, 0, 2).reshape(128, 4 * KD), dtype=f
        ),
        "b0": np.ascontiguousarray(b0.reshape(4, 128).T, dtype=f),
        "b1": np.ascontiguousarray(b1.reshape(4, 128).T, dtype=f),
        "b2": np.ascontiguousarray(b2.reshape(4, 128).T, dtype=f),
        "bq": np.ascontiguousarray((bq / 8.0).reshape(KD, 1), dtype=f),
        "keys_fm": np.ascontiguousarray(keys.T, dtype=f),
    }
    x_fm = np.ascontiguousarray(x.T, dtype=f)      # [128, B]
    temb_fm = np.ascontiguousarray(temb.T, dtype=f)  # [32, B]

    in_maps = []
    for c in range(NCORES):
        sl = slice(c * BC, (c + 1) * BC)
        m = dict(com)
        m["x_fm"] = np.ascontiguousarray(x_fm[:, sl])
        m["temb_fm"] = np.ascontiguousarray(temb_fm[:, sl])
        in_maps.append(m)
    return in_maps


def kernel(x, t, W0, b0, W1, b1, W2, b2, Wq, bq, keys, _profile=False):
    if "nc" not in _CACHE:
        _CACHE["nc"] = _build_program()
    nc = _CACHE["nc"]

    in_maps = _prep_inputs(
        np.asarray(x), np.asarray(t), np.asarray(W0), np.asarray(b0),
        np.asarray(W1), np.asarray(b1), np.asarray(W2), np.asarray(b2),
        np.asarray(Wq), np.asarray(bq), np.asarray(keys),
    )
    res = run_bass_kernel_spmd(
        nc, in_maps, list(range(NCORES)), trace=_profile
    )
    results = res.results

    weights = np.empty((B, TOPK), np.float32)
    top_idx = np.empty((B, TOPK), np.int32)
    for c in range(NCORES):
        r = results[c]
        sl = slice(c * BC, (c + 1) * BC)
        weights[sl] = r["weights_o"]
        pos = r["pos_o"].astype(np.int64)          # [BC, 64] slot in candidate array
        iw = r["iw_o"].astype(np.int64)            # [BC, 1024] within-tile idx
        tile_base = (pos >> 3) * TS
        within = np.take_along_axis(iw, pos, axis=1)
        top_idx[sl] = (tile_base + within).astype(np.int32)
    if _profile:
        return (weights, top_idx), res
    return weights, top_idx


# revision 8
# speedup vs baseline: 1.0888x; 1.0888x over previous
"""Trainium2 Bass kernel for AttentionWeightNet (MLP -> attention logits -> top-64 -> softmax).

Strategy (8 NeuronCores, data-parallel over batch):
  - Each core handles 512 of the 4096 batch rows. MLP params + 16MB key bank replicated.
  - Everything feature-major on device: activations stored [features, batch] so matmul
    contraction (K) sits on partitions and weights are naturally [K, M] (= lhsT).
  - Logits tiles [128 rows x 512 keys] are produced by PE into PSUM; the DVE max8
    instruction extracts each tile's top-8 values and max_index their within-tile
    positions (top-64 of a row provably lives in the per-512-chunk top-8 unless a
    single 512-chunk holds >=9 of the row's top-64 -- probability ~2e-3 per full
    problem instance, verified false on the reference input distribution).
  - Stage 2: per row-block, 8 rounds of (max8 + match_replace) over the 1024
    candidates give the sorted top-64 values; max_index recovers candidate slots;
    softmax on ACT+DVE. Host maps (slot -> tile, within-tile idx) to global indices
    (pure layout gather, no arithmetic).
"""

import math
import numpy as np

import concourse.bass as bass
import concourse.mybir as mybir
import concourse.tile as tile
from concourse import bacc
from concourse.bass_utils import run_bass_kernel_spmd

NCORES = 8
B, XD, TD, HID, KD, N, TOPK = 4096, 128, 32, 512, 64, 65536, 64
BC = B // NCORES          # rows per core
NBLK = BC // 128          # row blocks of 128 per core
TS = 512                  # keys per logits tile
NT = N // TS              # logits tiles per row block
KCH = 4096                # keys per DMA chunk
NKC = N // KCH            # key chunks
CAND = NT * 8             # stage-2 candidates per row
SELU_ALPHA = 1.6732632423543772
SELU_SCALE = 1.0507009873554805

F32 = mybir.dt.float32
U16 = mybir.dt.uint16
AF = mybir.ActivationFunctionType
ALU = mybir.AluOpType
NEG_BIG = -1.0e30

_CACHE = {}


def _build_program():
    nc = bacc.Bacc(
        "TRN2",
        target_bir_lowering=False,
        debug=False,
        enable_asserts=False,
        num_devices=NCORES,
    )

    def din(name, shape, dt=F32):
        return nc.dram_tensor(name, shape, dt, kind="ExternalInput").ap()

    def dout(name, shape, dt=F32):
        return nc.dram_tensor(name, shape, dt, kind="ExternalOutput").ap()

    ins = {
        "x_fm": din("x_fm", [XD, BC]),
        "temb_fm": din("temb_fm", [TD, BC]),
        "w0x": din("w0x", [XD, HID]),
        "w0t": din("w0t", [TD, HID]),
        "w1": din("w1", [128, 4 * HID]),
        "w2": din("w2", [128, 4 * HID]),
        "wq": din("wq", [128, 4 * KD]),
        "b0": din("b0", [128, 4]),
        "b1": din("b1", [128, 4]),
        "b2": din("b2", [128, 4]),
        "bq": din("bq", [KD, 1]),
        "keys_fm": din("keys_fm", [KD, N]),
    }
    outs = {
        "weights_o": dout("weights_o", [BC, TOPK]),
        "pos_o": dout("pos_o", [BC, TOPK], U16),
        "iw_o": dout("iw_o", [BC, CAND], U16),
    }

    with tile.TileContext(nc) as tc:
        _emit(tc, ins, outs)
    nc.compile()
    return nc


def _emit(tc, ins, outs):
    nc = tc.nc

    cp = tc.alloc_tile_pool(name="const", bufs=1)
    wp = tc.alloc_tile_pool(name="work", bufs=2)
    sp = tc.alloc_tile_pool(name="stage2", bufs=2)
    keys_pool = tc.alloc_tile_pool(name="keys", bufs=2)
    ev_pool = tc.alloc_tile_pool(name="evict", bufs=8)

    def load_const(name, shape, dt=F32):
        t = cp.tile(shape, dt, tag=name)
        nc.sync.dma_start(t[:], ins[name])
        return t

    x_fm = load_const("x_fm", [XD, BC])
    temb = load_const("temb_fm", [TD, BC])
    w0x = load_const("w0x", [XD, HID])
    w0t = load_const("w0t", [TD, HID])
    w1 = load_const("w1", [128, 4 * HID])
    w2 = load_const("w2", [128, 4 * HID])
    wq = load_const("wq", [128, 4 * KD])
    b0 = load_const("b0", [128, 4])
    b1 = load_const("b1", [128, 4])
    b2 = load_const("b2", [128, 4])
    bq = load_const("bq", [KD, 1])

    h1 = cp.tile([128, 4 * BC], F32, tag="h1")
    h2 = cp.tile([128, 4 * BC], F32, tag="h2")
    h3 = cp.tile([128, 4 * BC], F32, tag="h3")
    q = cp.tile([KD, BC], F32, tag="q")

    # ---- MLP (feature-major) ----
    with tc.tile_pool(name="psum_mlp", bufs=2, space="PSUM") as pmlp:

        def selu_from_psum(ps, out_ap, b_col):
            # selu(x+b) with x in PSUM; s*(x if x>0 else alpha*(exp(x)-1))
            e = wp.tile([128, BC], F32, tag="selu_e")
            xb = wp.tile([128, BC], F32, tag="selu_x")
            nc.scalar.activation(e[:], ps, AF.Exp, bias=b_col, scale=1.0)
            nc.scalar.activation(xb[:], ps, AF.Identity, bias=b_col, scale=1.0)
            # e <- s*alpha*(min(e,1) - 1)
            nc.vector.tensor_scalar(
                e[:], e[:], 1.0, None, op0=ALU.min
            )
            nc.vector.tensor_scalar(
                e[:], e[:], SELU_SCALE * SELU_ALPHA, -SELU_SCALE * SELU_ALPHA,
                op0=ALU.mult, op1=ALU.add,
            )
            # xb <- s*max(xb, 0)
            nc.vector.tensor_scalar(
                xb[:], xb[:], 0.0, SELU_SCALE, op0=ALU.max, op1=ALU.mult
            )
            nc.vector.tensor_add(out_ap, xb[:], e[:])

        # layer 1: in = [x (K=128); temb (K=32)]
        for m in range(4):
            ps = pmlp.tile([128, BC], F32, tag="mlp")
            nc.tensor.matmul(
                ps[:], w0x[:, m * 128:(m + 1) * 128], x_fm[:], start=True, stop=False
            )
            nc.tensor.matmul(
                ps[:], w0t[:, m * 128:(m + 1) * 128], temb[:], start=False, stop=True
            )
            selu_from_psum(ps[:], h1[:, m * BC:(m + 1) * BC], b0[:, m:m + 1])

        # layers 2 and 3: K = 512 in 4 chunks
        for (win, bin_, hin, hout) in ((w1, b1, h1, h2), (w2, b2, h2, h3)):
            for m in range(4):
                ps = pmlp.tile([128, BC], F32, tag="mlp")
                for k in range(4):
                    nc.tensor.matmul(
                        ps[:],
                        win[:, k * HID + m * 128: k * HID + (m + 1) * 128],
                        hin[:, k * BC:(k + 1) * BC],
                        start=(k == 0),
                        stop=(k == 3),
                    )
                selu_from_psum(ps[:], hout[:, m * BC:(m + 1) * BC], bin_[:, m:m + 1])

        # q = (h3 @ Wq + bq) / 8   (logit scale folded in; /8 is exact)
        psq = pmlp.tile([KD, BC], F32, tag="qp")
        for k in range(4):
            nc.tensor.matmul(
                psq[:],
                wq[:, k * KD:(k + 1) * KD],
                h3[:, k * BC:(k + 1) * BC],
                start=(k == 0),
                stop=(k == 3),
            )
        nc.scalar.activation(q[:], psq[:], AF.Identity, bias=bq[:, 0:1], scale=0.125)

    # ---- logits + per-tile top-8 screening ----
    W = [cp.tile([128, CAND], F32, name=f"W{b}", tag=f"W{b}") for b in range(NBLK)]
    Iw = [cp.tile([128, CAND], U16, name=f"I{b}", tag=f"I{b}") for b in range(NBLK)]

    with tc.tile_pool(name="psum_lg", bufs=8, space="PSUM") as plg:
        for kc in range(NKC):
            kt = keys_pool.tile([KD, KCH], F32, tag="keys")
            nc.sync.dma_start(kt[:], ins["keys_fm"][:, kc * KCH:(kc + 1) * KCH])
            for blk in range(NBLK):
                lhsT = q[:, blk * 128:(blk + 1) * 128]
                for t in range(KCH // TS):
                    ti = kc * (KCH // TS) + t
                    ps = plg.tile([128, TS], F32, tag="lg")
                    nc.tensor.matmul(
                        ps[:], lhsT, kt[:, t * TS:(t + 1) * TS], start=True, stop=True
                    )
                    # ACT (idle here) evicts PSUM->SBUF so both DVE scans run
                    # at the cheaper SBUF rate (58 vs 120 cyc overhead).
                    ev = ev_pool.tile([128, TS], F32, tag="ev")
                    nc.scalar.activation(ev[:], ps[:], AF.Copy, bias=0.0, scale=1.0)
                    wsl = W[blk][:, ti * 8:(ti + 1) * 8]
                    nc.vector.max(wsl, ev[:])
                    nc.vector.max_index(Iw[blk][:, ti * 8:(ti + 1) * 8], wsl, ev[:])

    # ---- stage 2: top-64 of the 1024 candidates, then softmax ----
    for blk in range(NBLK):
        Wo = W[blk]
        Wk = sp.tile([128, CAND], F32, tag="wwork")
        R = sp.tile([128, TOPK], F32, tag="rvals")
        for r in range(8):
            src = Wo[:] if r == 0 else Wk[:]
            rsl = R[:, r * 8:(r + 1) * 8]
            nc.vector.max(rsl, src)
            if r < 7:  # nothing reads Wk after the final round
                nc.vector.match_replace(Wk[:], rsl, src, NEG_BIG)
        pos = sp.tile([128, TOPK], U16, tag="pos")
        for r in range(8):
            nc.vector.max_index(pos[:, r * 8:(r + 1) * 8], R[:, r * 8:(r + 1) * 8], Wo[:])

        negm = sp.tile([128, 1], F32, tag="negm")
        nc.vector.tensor_scalar(negm[:], R[:, 0:1], -1.0, None, op0=ALU.mult)
        e64 = sp.tile([128, TOPK], F32, tag="e64")
        nc.scalar.activation(e64[:], R[:], AF.Exp, bias=negm[:, 0:1], scale=1.0)
        ssum = sp.tile([128, 1], F32, tag="ssum")
        nc.vector.reduce_sum(ssum[:], e64[:], axis=mybir.AxisListType.X)
        rec = sp.tile([128, 1], F32, tag="rec")
        nc.vector.reciprocal(rec[:], ssum[:])
        wts = sp.tile([128, TOPK], F32, tag="wts")
        nc.vector.tensor_mul(wts[:], e64[:], rec[:, 0:1].to_broadcast([128, TOPK]))

        rows = slice(blk * 128, (blk + 1) * 128)
        nc.sync.dma_start(outs["weights_o"][rows, :], wts[:])
        nc.sync.dma_start(outs["pos_o"][rows, :], pos[:])
        nc.sync.dma_start(outs["iw_o"][rows, :], Iw[blk][:])

    for pool in (ev_pool, keys_pool, sp, wp, cp):
        pool.release()


def _prep_inputs(x, t, W0, b0, W1, b1, W2, b2, Wq, bq, keys):
    """Host-side layout marshaling (transposes/reshapes only; temb is the one
    tiny precompute: 4096x16 sin/cos)."""
    f = np.float32
    half = TD // 2
    freqs = np.exp(-math.log(10000.0) * np.arange(half, dtype=np.float64) / half)
    args = t.astype(np.float64)[:, None] * freqs[None, :]
    temb = np.concatenate([np.sin(args), np.cos(args)], axis=-1).astype(f)  # [B, 32]

    com = {
        "w0x": np.ascontiguousarray(W0[:XD, :], dtype=f),
        "w0t": np.ascontiguousarray(W0[XD:, :], dtype=f),
        "w1": np.ascontiguousarray(
            W1.reshape(4, 128, HID).transpose(1, 0, 2).reshape(128, 4 * HID), dtype=f
        ),
        "w2": np.ascontiguousarray(
            W2.reshape(4, 128, HID).transpose(1, 0, 2).reshape(128, 4 * HID), dtype=f
        ),
        "wq": np.ascontiguousarray(
            Wq.reshape(4, 128, KD).transpose(1, 0, 2).reshape(128, 4 * KD), dtype=f
        ),
        "b0": np.ascontiguousarray(b0.reshape(4, 128).T, dtype=f),
        "b1": np.ascontiguousarray(b1.reshape(4, 128).T, dtype=f),
        "b2": np.ascontiguousarray(b2.reshape(4, 128).T, dtype=f),
        "bq": np.ascontiguousarray((bq / 8.0).reshape(KD, 1), dtype=f),
        "keys_fm": np.ascontiguousarray(keys.T, dtype=f),
    }
    x_fm = np.ascontiguousarray(x.T, dtype=f)      # [128, B]
    temb_fm = np.ascontiguousarray(temb.T, dtype=f)  # [32, B]

    in_maps = []
    for c in range(NCORES):
        sl = slice(c * BC, (c + 1) * BC)
        m = dict(com)
        m["x_fm"] = np.ascontiguousarray(x_fm[:, sl])
        m["temb_fm"] = np.ascontiguousarray(temb_fm[:, sl])
        in_maps.append(m)
    return in_maps


def kernel(x, t, W0, b0, W1, b1, W2, b2, Wq, bq, keys, _profile=False):
    if "nc" not in _CACHE:
        _CACHE["nc"] = _build_program()
    nc = _CACHE["nc"]

    in_maps = _prep_inputs(
        np.asarray(x), np.asarray(t), np.asarray(W0), np.asarray(b0),
        np.asarray(W1), np.asarray(b1), np.asarray(W2), np.asarray(b2),
        np.asarray(Wq), np.asarray(bq), np.asarray(keys),
    )
    res = run_bass_kernel_spmd(
        nc, in_maps, list(range(NCORES)), trace=_profile
    )
    results = res.results

    weights = np.empty((B, TOPK), np.float32)
    top_idx = np.empty((B, TOPK), np.int32)
    for c in range(NCORES):
        r = results[c]
        sl = slice(c * BC, (c + 1) * BC)
        weights[sl] = r["weights_o"]
        pos = r["pos_o"].astype(np.int64)          # [BC, 64] slot in candidate array
        iw = r["iw_o"].astype(np.int64)            # [BC, 1024] within-tile idx
        tile_base = (pos >> 3) * TS
        within = np.take_along_axis(iw, pos, axis=1)
        top_idx[sl] = (tile_base + within).astype(np.int32)
    if _profile:
        return (weights, top_idx), res
    return weights, top_idx
